# revision 13
# baseline (speedup 1.0000x reference)
"""Trainium2 Bass kernel for nn_MemoryAggregator (GNN attention aggregation).

Reference computation:
    Q = X@Wq; K = X@Wk; V = X@Wv            (X [100000,256], W [256,32])
    scores_e = <Q[src_e], K[dst_e]> / sqrt(32)   over 1.6M edges
    out[n]   = softmax-weighted sum over n's edges of V[dst_e]   ([100000,32])

Strategy (8 NeuronCores, SPMD, edges sharded by src node range):
  kernel1: per-core QKV projections of the core's 12500-node X shard.
           W^T stationary on the PE ([128,2,96] fp16), X^T streamed in 5
           chunks (2500 cols each, 5 psum tiles of 500); output written
           TRANSPOSED as qkv [96, 12500] fp16. DMA-bound (~8.8MB/core).
  host:    arrange per-edge K rows + per-quad q rows (fp16, one merged
           stream) and quad-interleaved V rows (bf16) into flat
           per-partition slot streams; each node's slots padded to a
           multiple of 4 (a "quad" shares one q row). Pad slots get
           K = -200*q/|q|^2 so their score is ~-35 and exp underflows.
  kernel2: per core, stream quad blocks sequentially and compute
           q*k -> half-add -> reduce (fp16, DVE), exp (ACT), ex*v +
           quad-sum (bf16, DVE); write quad numerators (bf16) and
           quad-summed exp (bf16). HBM-bandwidth-bound (~160B/edge).
  host:    per-node segment reduction of quad partials + division.

Softmax max-subtraction is dropped: scores/sqrt(32) ~ N(0,4), max over 1.6M
edges ~21, exp safe in f32/bf16 (fp16 scores, bf16 exp pipeline validated
well under the 2e-2 tolerance).
"""
import math
from contextlib import ExitStack

import numpy as np

import concourse.bass as bass
import concourse.tile as tile
from concourse import bacc, mybir
from concourse.bass_utils import run_bass_kernel_spmd

# ---------------------------------------------------------------- dimensions
N = 100000
E = 1600000
D_IN = 256
H = 32
DK = math.sqrt(H)
NCORES = 8
NPC = N // NCORES          # 12500 nodes per core
P = 128
G = 4                      # slots per group (one q row / numerator per group)
B = 240                    # slot block width per k2 step (multiple of G)

BF16 = mybir.dt.np(mybir.dt.bfloat16)

_cache = {}
LAST_TIMES = {}
LAST_S = None


# ================================================================ host prep
def _prep_core(src_l, dst):
    order = np.argsort(src_l, kind="stable")
    dst_s = dst[order].astype(np.int32)

    d = np.bincount(src_l, minlength=NPC)
    v = (d + G - 1) // G
    s = G * v

    # partition assignment: serpentine over nodes sorted by size desc
    node_order = np.argsort(-s, kind="stable")
    i = np.arange(NPC)
    pos = i % P
    pserp = np.where((i // P) % 2 == 0, pos, P - 1 - pos)
    part = np.empty(NPC, np.int64)
    part[node_order] = pserp
    load = np.bincount(part, weights=s, minlength=P).astype(np.int64)
    return {"d": d, "v": v, "s": s, "part": part, "dst_s": dst_s,
            "S_core": int(load.max())}


def _finalize_core(cc, S):
    d, v, s, part = cc["d"], cc["v"], cc["s"], cc["part"]

    perm = np.lexsort((np.arange(NPC), part))
    part_sorted = part[perm]
    sizes = s[perm]
    cs = np.cumsum(sizes) - sizes
    pstart = np.searchsorted(part_sorted, np.arange(P))
    base_at = cs[np.minimum(pstart, NPC - 1)]
    within = cs - base_at[part_sorted]
    within_node = np.empty(NPC, np.int64)
    within_node[perm] = within

    slotdst = np.full((P, S), -1, np.int32)
    nodes_rep = np.repeat(np.arange(NPC), d)
    ranks = np.arange(int(d.sum())) - np.repeat(np.cumsum(d) - d, d)
    cols = within_node[nodes_rep] + ranks
    slotdst[part[nodes_rep], cols] = cc["dst_s"]

    qvnode = np.full((P, S // G), -1, np.int32)
    vrep = np.repeat(np.arange(NPC), v)
    vranks = np.arange(int(v.sum())) - np.repeat(np.cumsum(v) - v, v)
    vcols = (within_node[vrep] // G) + vranks
    qvnode[part[vrep], vcols] = vrep

    cc["slotdst"] = slotdst
    cc["qvnode"] = qvnode
    del cc["dst_s"], cc["d"], cc["v"], cc["s"], cc["part"]


def _prep(edge_index):
    src = np.asarray(edge_index[0], dtype=np.int64)
    dst = np.asarray(edge_index[1], dtype=np.int64)
    core = src // NPC
    cores = []
    for c in range(NCORES):
        m = core == c
        cores.append(_prep_core(src[m] - c * NPC, dst[m]))
    S = max(cc["S_core"] for cc in cores)
    S = (S + G - 1) // G * G
    for cc in cores:
        _finalize_core(cc, S)
    return cores, S


# ================================================================ kernel 1
K1CH = 5                   # X^T chunks (overlap DMA with matmul)
K1T = 5                    # psum tiles per chunk
K1C = NPC // (K1CH * K1T)  # 500 columns per psum tile


def _build_k1(reps=1, bench_outs=False, out_rot=None, mode="full"):
    # bench mode: per-rep input shift (defeats CSE) + rotating live outputs
    OR = (min(reps, out_rot) if out_rot else reps) if bench_outs else 1
    SH = 2 if bench_outs else 0
    nc = bacc.Bacc("TRN2", target_bir_lowering=False)
    xt = nc.dram_tensor(
        "xt", [D_IN, NPC + SH * reps], mybir.dt.float16, kind="ExternalInput"
    )
    w = nc.dram_tensor("w", [D_IN, 3 * H], mybir.dt.float16, kind="ExternalInput")
    qkv = nc.dram_tensor(
        "qkv", [3 * H, OR * NPC], mybir.dt.float16, kind="ExternalOutput"
    )

    csz = NPC // K1CH
    with tile.TileContext(nc) as tc:
        with ExitStack() as ctx:
            wp = ctx.enter_context(tc.tile_pool(name="wp", bufs=1))
            xp = ctx.enter_context(tc.tile_pool(name="xp", bufs=4))
            pp = ctx.enter_context(tc.tile_pool(name="pp", bufs=4, space="PSUM"))
            op = ctx.enter_context(tc.tile_pool(name="op", bufs=2))
            w01 = wp.tile([P, 2, 3 * H], mybir.dt.float16, tag="w01")
            nc.sync.dma_start(w01[:], w.rearrange("(g p) e -> p g e", g=2))
            xc0 = None
            if mode == "compute":
                # one resident chunk; every rep recomputes from it
                xc0 = wp.tile([P, 2, csz], mybir.dt.float16, tag="xc0")
                nc.sync.dma_start(
                    xc0[:], xt[:, 0:csz].rearrange("(g p) n -> p g n", g=2)
                )
            for rep in range(reps):
                i0 = SH * rep
                o0 = (rep % OR) * NPC if bench_outs else 0
                obuf = op.tile([3 * H, NPC], mybir.dt.float16, tag="obuf")
                if mode == "dma":
                    nc.vector.memset(obuf[:, 0:1], 0.0)
                for ch in range(K1CH):
                    c0 = ch * csz
                    if mode == "compute":
                        xc = xc0
                    else:
                        # partition p holds X^T rows p and p+128 of the chunk
                        xc = xp.tile([P, 2, csz], mybir.dt.float16, tag="xc")
                        nc.sync.dma_start(
                            xc[:],
                            xt[:, i0 + c0 : i0 + c0 + csz].rearrange(
                                "(g p) n -> p g n", g=2
                            ),
                        )
                        if mode == "dma":
                            continue
                    for t in range(K1T):
                        r0 = t * K1C
                        ps = pp.tile([3 * H, K1C], mybir.dt.float32, tag="ps")
                        nc.tensor.matmul(
                            ps[:], w01[:, 0, :], xc[:, 0, r0 : r0 + K1C],
                            start=True, stop=False,
                        )
                        nc.tensor.matmul(
                            ps[:], w01[:, 1, :], xc[:, 1, r0 : r0 + K1C],
                            start=False, stop=True,
                        )
                        dst = obuf[:, c0 + r0 : c0 + r0 + K1C]
                        if t % 2 == 0:
                            nc.vector.tensor_copy(dst, ps[:])
                        else:
                            nc.scalar.activation(
                                dst, ps[:], mybir.ActivationFunctionType.Copy
                            )
                    if mode != "dma":
                        # per-chunk store on the ACT queue overlaps the next
                        # chunk's load on the SP queue
                        nc.scalar.dma_start(
                            qkv[:, o0 + c0 : o0 + c0 + csz],
                            obuf[:, c0 : c0 + csz],
                        )
                if mode == "dma":
                    nc.sync.dma_start(qkv[:, o0 : o0 + NPC], obuf[:])
    nc.compile()
    return nc


# ================================================================ kernel 2
def _build_k2(S, reps=1, wv_eng="vector", num_eng="pool", bench_outs=False,
              blk=None, sc_path="quarter", num_path="tt2", dma_split=True,
              mode="full", out_rot=None):
    Bw = blk or B
    NV = S // G
    # bench mode: per-rep input shift (defeats CSE) + rotating live outputs
    OR = (min(reps, out_rot) if out_rot else reps) if bench_outs else 1
    SH = 2 if bench_outs else 0
    nc = bacc.Bacc("TRN2", target_bir_lowering=False)
    kqs = nc.dram_tensor(
        "kqs", [P, NV + SH * reps, G + 1, H], mybir.dt.float16,
        kind="ExternalInput"
    )
    vsi = nc.dram_tensor(
        "vsi", [P, NV + SH * reps, H, G], mybir.dt.bfloat16, kind="ExternalInput"
    )
    outn = nc.dram_tensor(
        "outn", [P, OR * NV, H], mybir.dt.bfloat16, kind="ExternalOutput"
    )
    outd = nc.dram_tensor(
        "outd", [P, OR * NV, 1], mybir.dt.bfloat16, kind="ExternalOutput"
    )

    with tile.TileContext(nc) as tc:
        with ExitStack() as ctx:
            kp = ctx.enter_context(tc.tile_pool(name="kp", bufs=2))
            vp = ctx.enter_context(tc.tile_pool(name="vp", bufs=2))
            sp = ctx.enter_context(tc.tile_pool(name="sp", bufs=2))
            op = ctx.enter_context(tc.tile_pool(name="op", bufs=3))
            zt = None
            if mode == "dma":
                zp = ctx.enter_context(tc.tile_pool(name="zp", bufs=1))
                zt = zp.tile([P, Bw // G, H], mybir.dt.bfloat16, tag="zt")
                nc.vector.memset(zt[:], 0.0)
            dmae = nc.scalar if dma_split else nc.sync
            for rep, a in [
                (r, a) for r in range(reps) for a in range(0, S, Bw)
            ]:
                w = min(Bw, S - a)
                nv = w // G
                vb = a // G
                ob = ((rep % OR) * NV if bench_outs else 0) + vb
                iv = vb + SH * rep
                if mode != "compute" or (rep == 0 and a == 0):
                    kqt = kp.tile([P, nv, G + 1, H], mybir.dt.float16, tag="kqt")
                    nc.sync.dma_start(kqt[:], kqs[:, iv : iv + nv, :, :])
                    vst = vp.tile([P, nv, H, G], mybir.dt.bfloat16, tag="vst")
                    dmae.dma_start(vst[:], vsi[:, iv : iv + nv, :, :])
                if mode == "dma":
                    nc.sync.dma_start(outn[:, ob : ob + nv, :], zt[:, 0:nv, :])
                    nc.sync.dma_start(
                        outd[:, ob : ob + nv, :],
                        zt[:, 0:nv, 0:1],
                    )
                    continue

                k4 = kqt[:, :, 0:G, :]
                qv4 = kqt[:, :, G : G + 1, :]

                # scores on DVE: q*k -> half-add -> reduce
                pr = sp.tile([P, nv, G, H], mybir.dt.float16, tag="pr")
                nc.vector.tensor_tensor(
                    out=pr[:],
                    in0=qv4.to_broadcast([P, nv, G, H]),
                    in1=k4,
                    op=mybir.AluOpType.mult,
                )
                ph = sp.tile([P, nv, G, H // 2], mybir.dt.float16, tag="ph")
                nc.vector.tensor_tensor(
                    out=ph[:],
                    in0=pr[:, :, :, 0 : H // 2],
                    in1=pr[:, :, :, H // 2 : H],
                    op=mybir.AluOpType.add,
                )
                if sc_path == "tt":
                    # full pairwise-add chain: TT adds run 2x, TensorReduce 1x
                    cur = ph
                    wdt = H // 2
                    while wdt > 1:
                        nxt = sp.tile([P, nv, G, wdt // 2], mybir.dt.float16,
                                      tag=f"pc{wdt}")
                        nc.vector.tensor_tensor(
                            out=nxt[:],
                            in0=cur[:, :, :, 0 : wdt // 2],
                            in1=cur[:, :, :, wdt // 2 : wdt],
                            op=mybir.AluOpType.add,
                        )
                        cur = nxt
                        wdt //= 2
                    sc = cur[:].rearrange("p v t o -> p v (t o)")
                else:
                    if sc_path == "quarter":
                        pq = sp.tile([P, nv, G, H // 4], mybir.dt.float16,
                                     tag="pq")
                        nc.vector.tensor_tensor(
                            out=pq[:],
                            in0=ph[:, :, :, 0 : H // 4],
                            in1=ph[:, :, :, H // 4 : H // 2],
                            op=mybir.AluOpType.add,
                        )
                        red_in = pq
                    else:
                        red_in = ph
                    sct = sp.tile([P, nv, G], mybir.dt.float16, tag="sc")
                    with nc.allow_low_precision(reason="fp16 scores, |s|<70"):
                        nc.vector.tensor_reduce(
                            out=sct[:], in_=red_in[:], axis=mybir.AxisListType.X,
                            op=mybir.AluOpType.add,
                        )
                    sc = sct[:]
                # exp on ACT
                ex = sp.tile([P, nv, G], mybir.dt.bfloat16, tag="ex")
                nc.scalar.activation(
                    ex[:], sc, mybir.ActivationFunctionType.Exp, scale=1.0 / DK
                )
                # quad-summed denominator via paired adds (2x) not reduce (1x)
                e2 = sp.tile([P, nv, 2], mybir.dt.bfloat16, tag="e2")
                nc.vector.tensor_tensor(
                    out=e2[:], in0=ex[:, :, 0:2], in1=ex[:, :, 2:4],
                    op=mybir.AluOpType.add,
                )
                ed = op.tile([P, nv, 1], mybir.dt.bfloat16, tag="ed")
                nc.vector.tensor_tensor(
                    out=ed[:], in0=e2[:, :, 0:1], in1=e2[:, :, 1:2],
                    op=mybir.AluOpType.add,
                )
                # weighted V + quad sum
                exb = (
                    ex[:]
                    .rearrange("p v (o t) -> p v o t", o=1)
                    .to_broadcast([P, nv, H, G])
                )
                wv = sp.tile([P, nv, H, G], mybir.dt.bfloat16, tag="wv")
                (nc.gpsimd if wv_eng == "pool" else nc.vector).tensor_tensor(
                    out=wv[:], in0=exb, in1=vst[:], op=mybir.AluOpType.mult
                )
                non = op.tile([P, nv, H], mybir.dt.bfloat16, tag="non")
                neng = nc.gpsimd if num_eng == "pool" else nc.vector
                if num_path == "reduce":
                    with nc.allow_low_precision(reason="bf16 numer partials"):
                        neng.tensor_reduce(
                            out=non[:].rearrange("p v (h o) -> p v h o", o=1),
                            in_=wv[:], axis=mybir.AxisListType.X,
                            op=mybir.AluOpType.add,
                        )
                else:
                    wp2 = sp.tile([P, nv, H, G // 2], mybir.dt.bfloat16, tag="wp2")
                    neng.tensor_tensor(
                        out=wp2[:],
                        in0=wv[:, :, :, 0 : G // 2],
                        in1=wv[:, :, :, G // 2 : G],
                        op=mybir.AluOpType.add,
                    )
                    neng.tensor_tensor(
                        out=non[:].rearrange("p v (h o) -> p v h o", o=1),
                        in0=wp2[:, :, :, 0:1],
                        in1=wp2[:, :, :, 1:2],
                        op=mybir.AluOpType.add,
                    )
                nc.sync.dma_start(outn[:, ob : ob + nv, :], non[:])
                nc.sync.dma_start(outd[:, ob : ob + nv, :], ed[:])
    nc.compile()
    return nc


# ================================================================ host build
def _build_streams(cc, S, Kh, Vb, Qloc):
    """Per-core slot streams: kqs [P,NV,5,32] fp16 (4 K rows + q row per
    quad), vsi [P,NV,32,4] bf16 (quad-interleaved V)."""
    NV = S // G
    slotdst = cc["slotdst"]
    qvnode = cc["qvnode"]
    real = slotdst >= 0

    kss = np.zeros((P, S, H), np.float16)
    kss[real] = Kh[slotdst[real]]
    vss = np.zeros((P, S, H), BF16)
    vss[real] = Vb[slotdst[real]]
    vsi = np.ascontiguousarray(
        vss.reshape(P, NV, G, H).transpose(0, 1, 3, 2)
    )

    # pads sharing a quad with a real node: poison-K so exp(score) ~ 0
    qvn4 = np.repeat(qvnode, G, axis=1)
    padm = (~real) & (qvn4 >= 0)
    if padm.any():
        q = Qloc[qvn4[padm]].astype(np.float32)
        kpad = (-200.0 / np.maximum((q * q).sum(1), 1e-9))[:, None] * q
        kss[padm] = kpad.astype(np.float16)

    kqs = np.zeros((P, NV, G + 1, H), np.float16)
    kqs[:, :, 0:G, :] = kss.reshape(P, NV, G, H)
    validv = qvnode >= 0
    kqs[:, :, G, :][validv] = Qloc[qvnode[validv]]
    return kqs, vsi


def _combine(cc, outn, outd):
    """Per-node segment reduction of quad partials; returns [NPC, H]."""
    qvnode = cc["qvnode"].ravel()
    valid = qvnode >= 0
    idx = qvnode[valid]
    num = outn.reshape(-1, H)[valid].astype(np.float32)
    den = outd.reshape(-1)[valid].astype(np.float32)
    # vnodes of one node are contiguous (one partition, consecutive columns)
    starts = np.flatnonzero(np.diff(idx, prepend=idx[0] - 1) != 0)
    accn = np.add.reduceat(num, starts, axis=0)
    accd = np.add.reduceat(den, starts)
    accd = np.where(accd == 0, 1.0, accd)
    out = np.zeros((NPC, H), np.float32)
    out[idx[starts]] = accn / accd[:, None]
    return out


# ================================================================ driver
def kernel(X, edge_index, Wq, Wk, Wv):
    X = np.ascontiguousarray(np.asarray(X, dtype=np.float32))
    Wq = np.asarray(Wq, dtype=np.float32)
    Wk = np.asarray(Wk, dtype=np.float32)
    Wv = np.asarray(Wv, dtype=np.float32)
    ei = np.asarray(edge_index)

    global LAST_S
    cores, S = _prep(ei)
    LAST_S = S

    # ---- kernel 1: projections
    if "k1" not in _cache:
        _cache["k1"] = _build_k1()
    k1 = _cache["k1"]
    w_cat = np.concatenate([Wq, Wk, Wv], axis=1).astype(np.float16)  # [256, 96]
    in1 = [
        {
            "xt": np.ascontiguousarray(X[c * NPC : (c + 1) * NPC].T).astype(
                np.float16
            ),
            "w": w_cat,
        }
        for c in range(NCORES)
    ]
    r1 = run_bass_kernel_spmd(k1, in1, core_ids=list(range(NCORES)))
    LAST_TIMES["k1"] = r1.exec_time_ns
    # qkv comes back transposed: [96, NPC] per core
    qkvT = [r1.results[c]["qkv"] for c in range(NCORES)]
    Kh = np.ascontiguousarray(
        np.concatenate([q[H : 2 * H, :].T for q in qkvT], axis=0)
    )  # [N, 32] fp16
    Vb = np.concatenate(
        [q[2 * H :, :].T for q in qkvT], axis=0
    ).astype(BF16)

    # ---- kernel 2: stream slots, edge compute, quad partials
    if ("k2", S) not in _cache:
        _cache[("k2", S)] = _build_k2(S)
    k2 = _cache[("k2", S)]
    in2 = []
    for c in range(NCORES):
        kqs, vsi = _build_streams(
            cores[c], S, Kh, Vb, np.ascontiguousarray(qkvT[c][:H, :].T)
        )
        in2.append({"kqs": kqs, "vsi": vsi})
    r2 = run_bass_kernel_spmd(k2, in2, core_ids=list(range(NCORES)))
    LAST_TIMES["k2"] = r2.exec_time_ns

    # ---- host combine
    out = np.empty((N, H), dtype=np.float32)
    for c in range(NCORES):
        out[c * NPC : (c + 1) * NPC] = _combine(
            cores[c], r2.results[c]["outn"], r2.results[c]["outd"]
        )
    return out


# revision 19
# speedup vs baseline: 1.2611x; 1.2611x over previous
"""Trainium2 Bass kernel for nn_MemoryAggregator (GNN attention aggregation).

Reference computation:
    Q = X@Wq; K = X@Wk; V = X@Wv            (X [100000,256], W [256,32])
    scores_e = <Q[src_e], K[dst_e]> / sqrt(32)   over 1.6M edges
    out[n]   = softmax-weighted sum over n's edges of V[dst_e]   ([100000,32])

Strategy (8 NeuronCores, SPMD, edges sharded by src node range):
  kernel1: per-core QKV projections of the core's 12500-node X shard.
           W^T stationary on the PE ([128,2,96] fp16), X^T streamed in 5
           chunks (2500 cols each, 5 psum tiles of 500); output written
           TRANSPOSED as qkv [96, 12500] fp16. DMA-bound (~8.8MB/core).
  host:    arrange per-edge K rows + per-quad q rows (fp16, one merged
           stream) and quad-interleaved V rows (bf16) into flat
           per-partition slot streams; each node's slots padded to a
           multiple of 4 (a "quad" shares one q row). Pad slots get
           K = -200*q/|q|^2 so their score is ~-35 and exp underflows.
  kernel2: per core, stream quad blocks sequentially and compute
           q*k -> half-add -> reduce (fp16, DVE), exp (ACT), ex*v +
           quad-sum (bf16, DVE); write quad numerators (bf16) and
           quad-summed exp (bf16). HBM-bandwidth-bound (~160B/edge).
  host:    per-node segment reduction of quad partials + division.

Softmax max-subtraction is dropped: scores/sqrt(32) ~ N(0,4), max over 1.6M
edges ~21, exp safe in f32/bf16 (fp16 scores, bf16 exp pipeline validated
well under the 2e-2 tolerance).
"""
import math
from contextlib import ExitStack

import numpy as np

import concourse.bass as bass
import concourse.tile as tile
from concourse import bacc, mybir
from concourse.bass_utils import run_bass_kernel_spmd

# ---------------------------------------------------------------- dimensions
N = 100000
E = 1600000
D_IN = 256
H = 32
DK = math.sqrt(H)
NCORES = 8
NPC = N // NCORES          # 12500 nodes per core
P = 128
G = 4                      # slots per group (one q row / numerator per group)
B = 240                    # slot block width per k2 step (multiple of G)

BF16 = mybir.dt.np(mybir.dt.bfloat16)

_cache = {}
LAST_TIMES = {}
LAST_S = None


# ================================================================ host prep
def _prep_core(src_l, dst):
    order = np.argsort(src_l, kind="stable")
    dst_s = dst[order].astype(np.int32)

    d = np.bincount(src_l, minlength=NPC)
    r = d % 4
    v4 = d // 4 + (r == 3)          # quad groups (d%4==3 rounds into a quad)
    v2 = ((r == 1) | (r == 2)).astype(np.int64)   # one tail pair
    sq = 4 * v4
    sp = 2 * v2
    s = sq + sp

    # greedy best-fit-decreasing on total slots per partition
    node_order = np.argsort(-s, kind="stable")
    load = np.zeros(P, np.int64)
    part = np.empty(NPC, np.int64)
    for n in node_order:
        p = int(np.argmin(load))
        part[n] = p
        load[p] += s[n]
    loadq = np.bincount(part, weights=sq, minlength=P)
    loadp = np.bincount(part, weights=sp, minlength=P)
    return {"d": d, "v4": v4, "v2": v2, "part": part, "dst_s": dst_s,
            "Sq_core": int(loadq.max()), "Sp_core": int(loadp.max())}


def _finalize_core(cc, Sq, Sp):
    d, v4, v2, part = cc["d"], cc["v4"], cc["v2"], cc["part"]
    perm = np.lexsort((np.arange(NPC), part))
    part_sorted = part[perm]
    pstart = np.searchsorted(part_sorted, np.arange(P))

    def offsets(sizes):
        szp = sizes[perm]
        cs = np.cumsum(szp) - szp
        base = cs[np.minimum(pstart, NPC - 1)]
        within = cs - base[part_sorted]
        w = np.empty(NPC, np.int64)
        w[perm] = within
        return w

    oq = offsets(4 * v4)
    op_ = offsets(2 * v2)
    qcap = np.minimum(d, 4 * v4)    # edges that land in the quad region

    nodes_rep = np.repeat(np.arange(NPC), d)
    ranks = np.arange(int(d.sum())) - np.repeat(np.cumsum(d) - d, d)
    inq = ranks < qcap[nodes_rep]
    slotdst_q = np.full((P, Sq), -1, np.int32)
    nq, rq = nodes_rep[inq], ranks[inq]
    slotdst_q[part[nq], oq[nq] + rq] = cc["dst_s"][inq]
    slotdst_p = np.full((P, Sp), -1, np.int32)
    npr, rp = nodes_rep[~inq], ranks[~inq]
    slotdst_p[part[npr], op_[npr] + rp - qcap[npr]] = cc["dst_s"][~inq]

    qvnode_q = np.full((P, Sq // 4), -1, np.int32)
    vrep = np.repeat(np.arange(NPC), v4)
    vranks = np.arange(int(v4.sum())) - np.repeat(np.cumsum(v4) - v4, v4)
    qvnode_q[part[vrep], oq[vrep] // 4 + vranks] = vrep
    qvnode_p = np.full((P, Sp // 2), -1, np.int32)
    wn = np.flatnonzero(v2)
    qvnode_p[part[wn], op_[wn] // 2] = wn

    cc["slotdst_q"] = slotdst_q
    cc["qvnode_q"] = qvnode_q
    cc["slotdst_p"] = slotdst_p
    cc["qvnode_p"] = qvnode_p
    del cc["dst_s"], cc["d"], cc["v4"], cc["v2"], cc["part"]


def _prep(edge_index):
    src = np.asarray(edge_index[0], dtype=np.int64)
    dst = np.asarray(edge_index[1], dtype=np.int64)
    core = src // NPC
    cores = []
    for c in range(NCORES):
        m = core == c
        cores.append(_prep_core(src[m] - c * NPC, dst[m]))
    Sq = max(cc["Sq_core"] for cc in cores)
    Sq = (Sq + 3) // 4 * 4
    Sp = max(max(cc["Sp_core"] for cc in cores), 2)
    Sp = (Sp + 1) // 2 * 2
    for cc in cores:
        _finalize_core(cc, Sq, Sp)
    return cores, Sq, Sp


# ================================================================ kernel 1
K1CH = 5                   # X^T chunks (overlap DMA with matmul)
K1T = 5                    # psum tiles per chunk
K1C = NPC // (K1CH * K1T)  # 500 columns per psum tile


def _build_k1(reps=1, bench_outs=False, out_rot=None, mode="full"):
    # bench mode: per-rep input shift (defeats CSE) + rotating live outputs
    OR = (min(reps, out_rot) if out_rot else reps) if bench_outs else 1
    SH = 2 if bench_outs else 0
    nc = bacc.Bacc("TRN2", target_bir_lowering=False)
    xt = nc.dram_tensor(
        "xt", [D_IN, NPC + SH * reps], mybir.dt.float16, kind="ExternalInput"
    )
    w = nc.dram_tensor("w", [D_IN, 3 * H], mybir.dt.float16, kind="ExternalInput")
    qkv = nc.dram_tensor(
        "qkv", [3 * H, OR * NPC], mybir.dt.float16, kind="ExternalOutput"
    )

    csz = NPC // K1CH
    with tile.TileContext(nc) as tc:
        with ExitStack() as ctx:
            wp = ctx.enter_context(tc.tile_pool(name="wp", bufs=1))
            xp = ctx.enter_context(tc.tile_pool(name="xp", bufs=4))
            pp = ctx.enter_context(tc.tile_pool(name="pp", bufs=4, space="PSUM"))
            op = ctx.enter_context(tc.tile_pool(name="op", bufs=2))
            w01 = wp.tile([P, 2, 3 * H], mybir.dt.float16, tag="w01")
            nc.sync.dma_start(w01[:], w.rearrange("(g p) e -> p g e", g=2))
            xc0 = None
            if mode == "compute":
                # one resident chunk; every rep recomputes from it
                xc0 = wp.tile([P, 2, csz], mybir.dt.float16, tag="xc0")
                nc.sync.dma_start(
                    xc0[:], xt[:, 0:csz].rearrange("(g p) n -> p g n", g=2)
                )
            for rep in range(reps):
                i0 = SH * rep
                o0 = (rep % OR) * NPC if bench_outs else 0
                obuf = op.tile([3 * H, NPC], mybir.dt.float16, tag="obuf")
                if mode == "dma":
                    nc.vector.memset(obuf[:, 0:1], 0.0)
                for ch in range(K1CH):
                    c0 = ch * csz
                    if mode == "compute":
                        xc = xc0
                    else:
                        # partition p holds X^T rows p and p+128 of the chunk
                        xc = xp.tile([P, 2, csz], mybir.dt.float16, tag="xc")
                        nc.sync.dma_start(
                            xc[:],
                            xt[:, i0 + c0 : i0 + c0 + csz].rearrange(
                                "(g p) n -> p g n", g=2
                            ),
                        )
                        if mode == "dma":
                            continue
                    for t in range(K1T):
                        r0 = t * K1C
                        ps = pp.tile([3 * H, K1C], mybir.dt.float32, tag="ps")
                        nc.tensor.matmul(
                            ps[:], w01[:, 0, :], xc[:, 0, r0 : r0 + K1C],
                            start=True, stop=False,
                        )
                        nc.tensor.matmul(
                            ps[:], w01[:, 1, :], xc[:, 1, r0 : r0 + K1C],
                            start=False, stop=True,
                        )
                        dst = obuf[:, c0 + r0 : c0 + r0 + K1C]
                        if t % 2 == 0:
                            nc.vector.tensor_copy(dst, ps[:])
                        else:
                            nc.scalar.activation(
                                dst, ps[:], mybir.ActivationFunctionType.Copy
                            )
                    if mode != "dma":
                        # per-chunk store on the ACT queue overlaps the next
                        # chunk's load on the SP queue
                        nc.scalar.dma_start(
                            qkv[:, o0 + c0 : o0 + c0 + csz],
                            obuf[:, c0 : c0 + csz],
                        )
                if mode == "dma":
                    nc.sync.dma_start(qkv[:, o0 : o0 + NPC], obuf[:])
    nc.compile()
    return nc


# ================================================================ kernel 2
def _build_k2(Sq, Sp, reps=1, bench_outs=False, blk=None, blkp=None,
              dma_split=True, mode="full", out_rot=None):
    """Two-phase edge kernel: quad groups (4 slots share one q row and one
    numerator) then tail pairs (2 slots). All reductions are pairwise
    tensor_tensor add chains (2x DVE mode); TensorReduce (1x) is avoided.
    Slots of one node are contiguous within a partition row."""
    Bw = blk or B
    Bp = blkp or (B // 2)
    NVq = Sq // 4
    NVp = Sp // 2
    NVT = NVq + NVp
    # bench mode: per-rep input shift (defeats CSE) + rotating live outputs
    OR = (min(reps, out_rot) if out_rot else reps) if bench_outs else 1
    SH = 2 if bench_outs else 0
    nc = bacc.Bacc("TRN2", target_bir_lowering=False)
    kqs = nc.dram_tensor(
        "kqs", [P, NVq + SH * reps, 5, H], mybir.dt.float16,
        kind="ExternalInput"
    )
    vsi = nc.dram_tensor(
        "vsi", [P, NVq + SH * reps, H, 4], mybir.dt.bfloat16,
        kind="ExternalInput"
    )
    kqp = nc.dram_tensor(
        "kqp", [P, NVp + SH * reps, 3, H], mybir.dt.float16,
        kind="ExternalInput"
    )
    vsp = nc.dram_tensor(
        "vsp", [P, NVp + SH * reps, H, 2], mybir.dt.bfloat16,
        kind="ExternalInput"
    )
    outn = nc.dram_tensor(
        "outn", [P, OR * NVT, H], mybir.dt.bfloat16, kind="ExternalOutput"
    )
    outd = nc.dram_tensor(
        "outd", [P, OR * NVT, 1], mybir.dt.bfloat16, kind="ExternalOutput"
    )

    with tile.TileContext(nc) as tc:
        with ExitStack() as ctx:
            kp = ctx.enter_context(tc.tile_pool(name="kp", bufs=2))
            vp = ctx.enter_context(tc.tile_pool(name="vp", bufs=2))
            sp = ctx.enter_context(tc.tile_pool(name="sp", bufs=2))
            op = ctx.enter_context(tc.tile_pool(name="op", bufs=3))
            zt = None
            if mode == "dma":
                zp = ctx.enter_context(tc.tile_pool(name="zp", bufs=1))
                zt = zp.tile([P, Bw // 4, H], mybir.dt.bfloat16, tag="zt")
                nc.vector.memset(zt[:], 0.0)
                ztd = zp.tile([P, Bw // 4, 1], mybir.dt.bfloat16, tag="ztd")
                nc.vector.memset(ztd[:], 0.0)
            dmae = nc.scalar if dma_split else nc.sync

            def emit(g, kq_d, vs_d, nv, iv, ob):
                """One block of nv g-slot groups."""
                if mode != "compute" or emit.first:
                    kqt = kp.tile([P, nv, g + 1, H], mybir.dt.float16,
                                  tag="kqt")
                    nc.sync.dma_start(kqt[:], kq_d[:, iv : iv + nv, :, :])
                    vst = vp.tile([P, nv, H, g], mybir.dt.bfloat16, tag="vst")
                    dmae.dma_start(vst[:], vs_d[:, iv : iv + nv, :, :])
                    emit.tiles = (kqt, vst)
                    emit.first = False
                else:
                    kqt, vst = emit.tiles
                if mode == "dma":
                    nc.sync.dma_start(outn[:, ob : ob + nv, :], zt[:, 0:nv, :])
                    nc.sync.dma_start(outd[:, ob : ob + nv, :], ztd[:, 0:nv, :])
                    return
                k_ap = kqt[:, :, 0:g, :]
                qv = kqt[:, :, g : g + 1, :]
                # scores: q*k then pairwise-add chain down to 1
                pr = sp.tile([P, nv, g, H], mybir.dt.float16, tag="pr")
                nc.vector.tensor_tensor(
                    out=pr[:, :, 0:g, :], in0=qv.to_broadcast([P, nv, g, H]),
                    in1=k_ap, op=mybir.AluOpType.mult,
                )
                cur, wdt = pr, H
                while wdt > 1:
                    nxt = sp.tile([P, nv, g, wdt // 2], mybir.dt.float16,
                                  tag=f"pc{wdt}")
                    nc.vector.tensor_tensor(
                        out=nxt[:, :, 0:g, :],
                        in0=cur[:, :, 0:g, 0 : wdt // 2],
                        in1=cur[:, :, 0:g, wdt // 2 : wdt],
                        op=mybir.AluOpType.add,
                    )
                    cur, wdt = nxt, wdt // 2
                sc = cur[:, :, 0:g, 0:1].rearrange("p v t o -> p v (t o)")
                ex = sp.tile([P, nv, g], mybir.dt.bfloat16, tag="ex")
                nc.scalar.activation(
                    ex[:, :, 0:g], sc, mybir.ActivationFunctionType.Exp,
                    scale=1.0 / DK,
                )
                # group-summed denominator (pairwise adds)
                if g == 4:
                    e2 = sp.tile([P, nv, 2], mybir.dt.bfloat16, tag="e2")
                    nc.vector.tensor_tensor(
                        out=e2[:], in0=ex[:, :, 0:2], in1=ex[:, :, 2:4],
                        op=mybir.AluOpType.add,
                    )
                    edi = e2
                else:
                    edi = ex
                ed = op.tile([P, nv, 1], mybir.dt.bfloat16, tag="ed")
                nc.vector.tensor_tensor(
                    out=ed[:], in0=edi[:, :, 0:1], in1=edi[:, :, 1:2],
                    op=mybir.AluOpType.add,
                )
                # weighted V + group sum
                exb = (
                    ex[:, :, 0:g]
                    .rearrange("p v (o t) -> p v o t", o=1)
                    .to_broadcast([P, nv, H, g])
                )
                wv = sp.tile([P, nv, H, g], mybir.dt.bfloat16, tag="wv")
                nc.vector.tensor_tensor(
                    out=wv[:, :, :, 0:g], in0=exb, in1=vst[:, :, :, 0:g],
                    op=mybir.AluOpType.mult,
                )
                if g == 4:
                    wp2 = sp.tile([P, nv, H, 2], mybir.dt.bfloat16, tag="wp2")
                    nc.vector.tensor_tensor(
                        out=wp2[:],
                        in0=wv[:, :, :, 0:2], in1=wv[:, :, :, 2:4],
                        op=mybir.AluOpType.add,
                    )
                    wfin = wp2
                else:
                    wfin = wv
                non = op.tile([P, nv, H], mybir.dt.bfloat16, tag="non")
                nc.vector.tensor_tensor(
                    out=non[:, :, :].rearrange("p v (h o) -> p v h o", o=1),
                    in0=wfin[:, :, :, 0:1], in1=wfin[:, :, :, 1:2],
                    op=mybir.AluOpType.add,
                )
                nc.sync.dma_start(outn[:, ob : ob + nv, :], non[:, :, :])
                nc.sync.dma_start(outd[:, ob : ob + nv, :], ed[:])

            emit.first = True
            emit.tiles = None
            for rep in range(reps):
                ob0 = (rep % OR) * NVT if bench_outs else 0
                for a in range(0, Sq, Bw):
                    w = min(Bw, Sq - a)
                    emit(4, kqs, vsi, w // 4, a // 4 + SH * rep,
                         ob0 + a // 4)
                for a in range(0, Sp, Bp):
                    w = min(Bp, Sp - a)
                    emit(2, kqp, vsp, w // 2, a // 2 + SH * rep,
                         ob0 + NVq + a // 2)
    nc.compile()
    return nc


# ================================================================ host build
def _stream_phase(slotdst, qvnode, g, Kh, Vb, Qloc):
    """One phase's streams: kq [P,NV,g+1,32] fp16 (g K rows + q row per
    group), vs [P,NV,32,g] bf16 (group-interleaved V)."""
    S = slotdst.shape[1]
    NV = S // g
    real = slotdst >= 0

    kss = np.zeros((P, S, H), np.float16)
    kss[real] = Kh[slotdst[real]]
    vss = np.zeros((P, S, H), BF16)
    vss[real] = Vb[slotdst[real]]
    vs = np.ascontiguousarray(vss.reshape(P, NV, g, H).transpose(0, 1, 3, 2))

    # pads sharing a group with a real node: poison-K so exp(score) ~ 0
    qvn = np.repeat(qvnode, g, axis=1)
    padm = (~real) & (qvn >= 0)
    if padm.any():
        q = Qloc[qvn[padm]].astype(np.float32)
        kpad = (-200.0 / np.maximum((q * q).sum(1), 1e-9))[:, None] * q
        kss[padm] = kpad.astype(np.float16)

    kq = np.zeros((P, NV, g + 1, H), np.float16)
    kq[:, :, 0:g, :] = kss.reshape(P, NV, g, H)
    validv = qvnode >= 0
    kq[:, :, g, :][validv] = Qloc[qvnode[validv]]
    return kq, vs


def _combine(cc, outn, outd):
    """Per-node segment reduction of group partials; returns [NPC, H].
    outn rows: [quad groups | pair groups] per partition."""
    qcat = np.concatenate([cc["qvnode_q"], cc["qvnode_p"]], axis=1).ravel()
    valid = qcat >= 0
    idx = qcat[valid]
    num = outn.reshape(-1, H)[valid].astype(np.float32)
    den = outd.reshape(-1)[valid].astype(np.float32)
    # groups of one node are contiguous within a region, but a node may have
    # a quad run AND a pair row -> accumulate run sums per node
    starts = np.flatnonzero(np.diff(idx, prepend=idx[0] - 1) != 0)
    accn = np.add.reduceat(num, starts, axis=0)
    accd = np.add.reduceat(den, starts)
    nsum = np.zeros((NPC, H), np.float32)
    dsum = np.zeros(NPC, np.float32)
    np.add.at(nsum, idx[starts], accn)
    np.add.at(dsum, idx[starts], accd)
    dsum[dsum == 0] = 1.0
    return nsum / dsum[:, None]


# ================================================================ driver
def kernel(X, edge_index, Wq, Wk, Wv):
    X = np.ascontiguousarray(np.asarray(X, dtype=np.float32))
    Wq = np.asarray(Wq, dtype=np.float32)
    Wk = np.asarray(Wk, dtype=np.float32)
    Wv = np.asarray(Wv, dtype=np.float32)
    ei = np.asarray(edge_index)

    global LAST_S
    cores, Sq, Sp = _prep(ei)
    LAST_S = (Sq, Sp)

    # ---- kernel 1: projections
    if "k1" not in _cache:
        _cache["k1"] = _build_k1()
    k1 = _cache["k1"]
    w_cat = np.concatenate([Wq, Wk, Wv], axis=1).astype(np.float16)  # [256, 96]
    in1 = [
        {
            "xt": np.ascontiguousarray(X[c * NPC : (c + 1) * NPC].T).astype(
                np.float16
            ),
            "w": w_cat,
        }
        for c in range(NCORES)
    ]
    r1 = run_bass_kernel_spmd(k1, in1, core_ids=list(range(NCORES)))
    LAST_TIMES["k1"] = r1.exec_time_ns
    # qkv comes back transposed: [96, NPC] per core
    qkvT = [r1.results[c]["qkv"] for c in range(NCORES)]
    Kh = np.ascontiguousarray(
        np.concatenate([q[H : 2 * H, :].T for q in qkvT], axis=0)
    )  # [N, 32] fp16
    Vb = np.concatenate(
        [q[2 * H :, :].T for q in qkvT], axis=0
    ).astype(BF16)

    # ---- kernel 2: stream slots, edge compute, group partials
    if ("k2", Sq, Sp) not in _cache:
        _cache[("k2", Sq, Sp)] = _build_k2(Sq, Sp)
    k2 = _cache[("k2", Sq, Sp)]
    in2 = []
    for c in range(NCORES):
        Qloc = np.ascontiguousarray(qkvT[c][:H, :].T)
        kqs, vsi = _stream_phase(
            cores[c]["slotdst_q"], cores[c]["qvnode_q"], 4, Kh, Vb, Qloc
        )
        kqp, vsp = _stream_phase(
            cores[c]["slotdst_p"], cores[c]["qvnode_p"], 2, Kh, Vb, Qloc
        )
        in2.append({"kqs": kqs, "vsi": vsi, "kqp": kqp, "vsp": vsp})
    r2 = run_bass_kernel_spmd(k2, in2, core_ids=list(range(NCORES)))
    LAST_TIMES["k2"] = r2.exec_time_ns

    # ---- host combine
    out = np.empty((N, H), dtype=np.float32)
    for c in range(NCORES):
        out[c * NPC : (c + 1) * NPC] = _combine(
            cores[c], r2.results[c]["outn"], r2.results[c]["outd"]
        )
    return out


# revision 24
# speedup vs baseline: 1.2679x; 1.0054x over previous
"""Trainium2 Bass kernel for nn_MemoryAggregator (GNN attention aggregation).

Reference computation:
    Q = X@Wq; K = X@Wk; V = X@Wv            (X [100000,256], W [256,32])
    scores_e = <Q[src_e], K[dst_e]> / sqrt(32)   over 1.6M edges
    out[n]   = softmax-weighted sum over n's edges of V[dst_e]   ([100000,32])

Strategy (8 NeuronCores, SPMD, edges sharded by src node range):
  kernel1: per-core QKV projections of the core's 12500-node X shard.
           W^T stationary on the PE ([128,2,96] fp16), X^T streamed in 5
           chunks (2500 cols each, 5 psum tiles of 500, per-chunk output
           stores on the ACT queue); output written TRANSPOSED as
           qkv [96, 12500] fp16. DMA-bound (~8.8MB/core, ~25us).
  host:    two-phase slot streams per partition row: QUADS (4 slots share
           one q row + one numerator pair) for d//4 groups per node, plus
           one tail PAIR for d%4 in {1,2} (d%4==3 rounds into a quad).
           Greedy best-fit-decreasing packs nodes onto the 128 partitions
           (~1636 slots/partition vs 1562.5 ideal). Pad slots get
           K = -200*q/|q|^2 so their score is ~-35 and exp underflows.
  kernel2: per core, stream blocks and compute on the DVE only (GPSIMD
           tensor ops measured 3-4x slower than the cost model; every
           offload attempt regressed): q*k products then a pairwise
           tensor_tensor add chain (all 2x mode; TensorReduce runs 1x and
           is avoided), exp on ACT, ex*v + pair-sum (bf16). Writes
           PER-PAIR numerators [H,2] (the last 1x add is pushed to the
           host) + group-summed exp. ~115 DVE-ns/slot vs ~105 DMA-ns.
  host:    per-node segment reduction (f32) of pair partials + division.

Softmax max-subtraction is dropped: scores/sqrt(32) ~ N(0,4), max over 1.6M
edges ~21, exp safe in f32/bf16 (fp16 scores, bf16 exp pipeline validated
well under the 2e-2 tolerance).

Measured on the axon trn2 pool (PJRT wall-clock rep-differencing, see
test.py): baseline 187.2us total -> k1 25.5us + k2 ~105us.
"""
import math
from contextlib import ExitStack

import numpy as np

import concourse.bass as bass
import concourse.tile as tile
from concourse import bacc, mybir
from concourse.bass_utils import run_bass_kernel_spmd

# ---------------------------------------------------------------- dimensions
N = 100000
E = 1600000
D_IN = 256
H = 32
DK = math.sqrt(H)
NCORES = 8
NPC = N // NCORES          # 12500 nodes per core
P = 128
G = 4                      # slots per group (one q row / numerator per group)
B = 240                    # slot block width per k2 step (multiple of G)

BF16 = mybir.dt.np(mybir.dt.bfloat16)

_cache = {}
LAST_TIMES = {}
LAST_S = None


# ================================================================ host prep
def _prep_core(src_l, dst):
    order = np.argsort(src_l, kind="stable")
    dst_s = dst[order].astype(np.int32)

    d = np.bincount(src_l, minlength=NPC)
    r = d % 4
    v4 = d // 4 + (r == 3)          # quad groups (d%4==3 rounds into a quad)
    v2 = ((r == 1) | (r == 2)).astype(np.int64)   # one tail pair
    sq = 4 * v4
    sp = 2 * v2
    s = sq + sp

    # greedy best-fit-decreasing on total slots per partition
    node_order = np.argsort(-s, kind="stable")
    load = np.zeros(P, np.int64)
    part = np.empty(NPC, np.int64)
    for n in node_order:
        p = int(np.argmin(load))
        part[n] = p
        load[p] += s[n]
    loadq = np.bincount(part, weights=sq, minlength=P)
    loadp = np.bincount(part, weights=sp, minlength=P)
    return {"d": d, "v4": v4, "v2": v2, "part": part, "dst_s": dst_s,
            "Sq_core": int(loadq.max()), "Sp_core": int(loadp.max())}


def _finalize_core(cc, Sq, Sp):
    d, v4, v2, part = cc["d"], cc["v4"], cc["v2"], cc["part"]
    perm = np.lexsort((np.arange(NPC), part))
    part_sorted = part[perm]
    pstart = np.searchsorted(part_sorted, np.arange(P))

    def offsets(sizes):
        szp = sizes[perm]
        cs = np.cumsum(szp) - szp
        base = cs[np.minimum(pstart, NPC - 1)]
        within = cs - base[part_sorted]
        w = np.empty(NPC, np.int64)
        w[perm] = within
        return w

    oq = offsets(4 * v4)
    op_ = offsets(2 * v2)
    qcap = np.minimum(d, 4 * v4)    # edges that land in the quad region

    nodes_rep = np.repeat(np.arange(NPC), d)
    ranks = np.arange(int(d.sum())) - np.repeat(np.cumsum(d) - d, d)
    inq = ranks < qcap[nodes_rep]
    slotdst_q = np.full((P, Sq), -1, np.int32)
    nq, rq = nodes_rep[inq], ranks[inq]
    slotdst_q[part[nq], oq[nq] + rq] = cc["dst_s"][inq]
    slotdst_p = np.full((P, Sp), -1, np.int32)
    npr, rp = nodes_rep[~inq], ranks[~inq]
    slotdst_p[part[npr], op_[npr] + rp - qcap[npr]] = cc["dst_s"][~inq]

    qvnode_q = np.full((P, Sq // 4), -1, np.int32)
    vrep = np.repeat(np.arange(NPC), v4)
    vranks = np.arange(int(v4.sum())) - np.repeat(np.cumsum(v4) - v4, v4)
    qvnode_q[part[vrep], oq[vrep] // 4 + vranks] = vrep
    qvnode_p = np.full((P, Sp // 2), -1, np.int32)
    wn = np.flatnonzero(v2)
    qvnode_p[part[wn], op_[wn] // 2] = wn

    cc["slotdst_q"] = slotdst_q
    cc["qvnode_q"] = qvnode_q
    cc["slotdst_p"] = slotdst_p
    cc["qvnode_p"] = qvnode_p
    del cc["dst_s"], cc["d"], cc["v4"], cc["v2"], cc["part"]


def _prep(edge_index):
    src = np.asarray(edge_index[0], dtype=np.int64)
    dst = np.asarray(edge_index[1], dtype=np.int64)
    core = src // NPC
    cores = []
    for c in range(NCORES):
        m = core == c
        cores.append(_prep_core(src[m] - c * NPC, dst[m]))
    Sq = max(cc["Sq_core"] for cc in cores)
    Sq = (Sq + 3) // 4 * 4
    Sp = max(max(cc["Sp_core"] for cc in cores), 2)
    Sp = (Sp + 1) // 2 * 2
    for cc in cores:
        _finalize_core(cc, Sq, Sp)
    return cores, Sq, Sp


# ================================================================ kernel 1
K1CH = 5                   # X^T chunks (overlap DMA with matmul)
K1T = 5                    # psum tiles per chunk
K1C = NPC // (K1CH * K1T)  # 500 columns per psum tile


def _build_k1(reps=1, bench_outs=False, out_rot=None, mode="full"):
    # bench mode: per-rep input shift (defeats CSE) + rotating live outputs
    OR = (min(reps, out_rot) if out_rot else reps) if bench_outs else 1
    SH = 2 if bench_outs else 0
    nc = bacc.Bacc("TRN2", target_bir_lowering=False)
    xt = nc.dram_tensor(
        "xt", [D_IN, NPC + SH * reps], mybir.dt.float16, kind="ExternalInput"
    )
    w = nc.dram_tensor("w", [D_IN, 3 * H], mybir.dt.float16, kind="ExternalInput")
    qkv = nc.dram_tensor(
        "qkv", [3 * H, OR * NPC], mybir.dt.float16, kind="ExternalOutput"
    )

    csz = NPC // K1CH
    with tile.TileContext(nc) as tc:
        with ExitStack() as ctx:
            wp = ctx.enter_context(tc.tile_pool(name="wp", bufs=1))
            xp = ctx.enter_context(tc.tile_pool(name="xp", bufs=4))
            pp = ctx.enter_context(tc.tile_pool(name="pp", bufs=4, space="PSUM"))
            op = ctx.enter_context(tc.tile_pool(name="op", bufs=2))
            w01 = wp.tile([P, 2, 3 * H], mybir.dt.float16, tag="w01")
            nc.sync.dma_start(w01[:], w.rearrange("(g p) e -> p g e", g=2))
            xc0 = None
            if mode == "compute":
                # one resident chunk; every rep recomputes from it
                xc0 = wp.tile([P, 2, csz], mybir.dt.float16, tag="xc0")
                nc.sync.dma_start(
                    xc0[:], xt[:, 0:csz].rearrange("(g p) n -> p g n", g=2)
                )
            for rep in range(reps):
                i0 = SH * rep
                o0 = (rep % OR) * NPC if bench_outs else 0
                obuf = op.tile([3 * H, NPC], mybir.dt.float16, tag="obuf")
                if mode == "dma":
                    nc.vector.memset(obuf[:, 0:1], 0.0)
                for ch in range(K1CH):
                    c0 = ch * csz
                    if mode == "compute":
                        xc = xc0
                    else:
                        # partition p holds X^T rows p and p+128 of the chunk
                        xc = xp.tile([P, 2, csz], mybir.dt.float16, tag="xc")
                        nc.sync.dma_start(
                            xc[:],
                            xt[:, i0 + c0 : i0 + c0 + csz].rearrange(
                                "(g p) n -> p g n", g=2
                            ),
                        )
                        if mode == "dma":
                            continue
                    for t in range(K1T):
                        r0 = t * K1C
                        ps = pp.tile([3 * H, K1C], mybir.dt.float32, tag="ps")
                        nc.tensor.matmul(
                            ps[:], w01[:, 0, :], xc[:, 0, r0 : r0 + K1C],
                            start=True, stop=False,
                        )
                        nc.tensor.matmul(
                            ps[:], w01[:, 1, :], xc[:, 1, r0 : r0 + K1C],
                            start=False, stop=True,
                        )
                        dst = obuf[:, c0 + r0 : c0 + r0 + K1C]
                        if t % 2 == 0:
                            nc.vector.tensor_copy(dst, ps[:])
                        else:
                            nc.scalar.activation(
                                dst, ps[:], mybir.ActivationFunctionType.Copy
                            )
                    if mode != "dma":
                        # per-chunk store on the ACT queue overlaps the next
                        # chunk's load on the SP queue
                        nc.scalar.dma_start(
                            qkv[:, o0 + c0 : o0 + c0 + csz],
                            obuf[:, c0 : c0 + csz],
                        )
                if mode == "dma":
                    nc.sync.dma_start(qkv[:, o0 : o0 + NPC], obuf[:])
    nc.compile()
    return nc


# ================================================================ kernel 2
def _build_k2(Sq, Sp, reps=1, bench_outs=False, blk=None, blkp=None,
              dma_split=True, mode="full", out_rot=None,
              pool_pairs=False, pool_qblocks=0, alias=False):
    """Two-phase edge kernel: quad groups (4 slots share one q row and one
    numerator) then tail pairs (2 slots). All reductions are pairwise
    tensor_tensor add chains (2x DVE mode); TensorReduce (1x) is avoided.
    Slots of one node are contiguous within a partition row."""
    Bw = blk or B
    Bp = blkp or (B // 2)
    NVq = Sq // 4
    NVp = Sp // 2
    NVT = NVq + NVp
    # bench mode: per-rep input shift (defeats CSE) + rotating live outputs
    OR = (min(reps, out_rot) if out_rot else reps) if bench_outs else 1
    SH = 2 if bench_outs else 0
    nc = bacc.Bacc("TRN2", target_bir_lowering=False)
    kqs = nc.dram_tensor(
        "kqs", [P, NVq + SH * reps, 5, H], mybir.dt.float16,
        kind="ExternalInput"
    )
    vsi = nc.dram_tensor(
        "vsi", [P, NVq + SH * reps, H, 4], mybir.dt.bfloat16,
        kind="ExternalInput"
    )
    kqp = nc.dram_tensor(
        "kqp", [P, NVp + SH * reps, 3, H], mybir.dt.float16,
        kind="ExternalInput"
    )
    vsp = nc.dram_tensor(
        "vsp", [P, NVp + SH * reps, H, 2], mybir.dt.bfloat16,
        kind="ExternalInput"
    )
    outn = nc.dram_tensor(
        "outn", [P, OR * NVT, H, 2], mybir.dt.bfloat16, kind="ExternalOutput"
    )
    outd = nc.dram_tensor(
        "outd", [P, OR * NVT, 1], mybir.dt.bfloat16, kind="ExternalOutput"
    )

    with tile.TileContext(nc) as tc:
        with ExitStack() as ctx:
            kp = ctx.enter_context(tc.tile_pool(name="kp", bufs=2))
            vp = ctx.enter_context(tc.tile_pool(name="vp", bufs=2))
            sp = ctx.enter_context(tc.tile_pool(name="sp", bufs=2))
            op = ctx.enter_context(tc.tile_pool(name="op", bufs=3))
            zt = None
            if mode == "dma":
                zp = ctx.enter_context(tc.tile_pool(name="zp", bufs=1))
                zt = zp.tile([P, Bw // 4, H, 2], mybir.dt.bfloat16, tag="zt")
                nc.vector.memset(zt[:], 0.0)
                ztd = zp.tile([P, Bw // 4, 1], mybir.dt.bfloat16, tag="ztd")
                nc.vector.memset(ztd[:], 0.0)
            dmae = nc.scalar if dma_split else nc.sync

            def emit(g, kq_d, vs_d, nv, iv, ob, veng=None):
                """One block of nv g-slot groups; veng picks the vector
                engine (DVE or GPSIMD) for the whole block's pipeline."""
                veng = veng or nc.vector
                if mode != "compute" or emit.first:
                    kqt = kp.tile([P, nv, g + 1, H], mybir.dt.float16,
                                  tag="kqt")
                    nc.sync.dma_start(kqt[:], kq_d[:, iv : iv + nv, :, :])
                    vst = vp.tile([P, nv, H, g], mybir.dt.bfloat16, tag="vst")
                    dmae.dma_start(vst[:], vs_d[:, iv : iv + nv, :, :])
                    emit.tiles = (kqt, vst)
                    emit.first = False
                else:
                    kqt, vst = emit.tiles
                if mode == "dma":
                    nc.sync.dma_start(
                        outn[:, ob : ob + nv, :, :], zt[:, 0:nv, :, :])
                    nc.sync.dma_start(outd[:, ob : ob + nv, :], ztd[:, 0:nv, :])
                    return
                k_ap = kqt[:, :, 0:g, :]
                qv = kqt[:, :, g : g + 1, :]
                # scores: q*k then pairwise-add chain down to 1
                if alias:
                    # in-place: products overwrite the K region (write region
                    # == read region, element-aligned: DVE reads precede the
                    # trailing writeback), chain collapses onto its low half
                    veng.tensor_tensor(
                        out=k_ap, in0=qv.to_broadcast([P, nv, g, H]),
                        in1=k_ap, op=mybir.AluOpType.mult,
                    )
                    wdt = H
                    while wdt > 1:
                        veng.tensor_tensor(
                            out=kqt[:, :, 0:g, 0 : wdt // 2],
                            in0=kqt[:, :, 0:g, 0 : wdt // 2],
                            in1=kqt[:, :, 0:g, wdt // 2 : wdt],
                            op=mybir.AluOpType.add,
                        )
                        wdt //= 2
                    sc = kqt[:, :, 0:g, 0:1].rearrange("p v t o -> p v (t o)")
                else:
                    pr = sp.tile([P, nv, g, H], mybir.dt.float16, tag="pr")
                    veng.tensor_tensor(
                        out=pr[:, :, 0:g, :],
                        in0=qv.to_broadcast([P, nv, g, H]),
                        in1=k_ap, op=mybir.AluOpType.mult,
                    )
                    cur, wdt = pr, H
                    while wdt > 1:
                        nxt = sp.tile([P, nv, g, wdt // 2], mybir.dt.float16,
                                      tag=f"pc{wdt}")
                        veng.tensor_tensor(
                            out=nxt[:, :, 0:g, :],
                            in0=cur[:, :, 0:g, 0 : wdt // 2],
                            in1=cur[:, :, 0:g, wdt // 2 : wdt],
                            op=mybir.AluOpType.add,
                        )
                        cur, wdt = nxt, wdt // 2
                    sc = cur[:, :, 0:g, 0:1].rearrange("p v t o -> p v (t o)")
                ex = sp.tile([P, nv, g], mybir.dt.bfloat16, tag="ex")
                nc.scalar.activation(
                    ex[:, :, 0:g], sc, mybir.ActivationFunctionType.Exp,
                    scale=1.0 / DK,
                )
                # group-summed denominator (pairwise adds)
                if g == 4:
                    e2 = sp.tile([P, nv, 2], mybir.dt.bfloat16, tag="e2")
                    veng.tensor_tensor(
                        out=e2[:], in0=ex[:, :, 0:2], in1=ex[:, :, 2:4],
                        op=mybir.AluOpType.add,
                    )
                    edi = e2
                else:
                    edi = ex
                ed = op.tile([P, nv, 1], mybir.dt.bfloat16, tag="ed")
                veng.tensor_tensor(
                    out=ed[:], in0=edi[:, :, 0:1], in1=edi[:, :, 1:2],
                    op=mybir.AluOpType.add,
                )
                # weighted V + group sum
                exb = (
                    ex[:, :, 0:g]
                    .rearrange("p v (o t) -> p v o t", o=1)
                    .to_broadcast([P, nv, H, g])
                )
                wv = sp.tile([P, nv, H, g], mybir.dt.bfloat16, tag="wv")
                veng.tensor_tensor(
                    out=wv[:, :, :, 0:g], in0=exb, in1=vst[:, :, :, 0:g],
                    op=mybir.AluOpType.mult,
                )
                if g == 4:
                    # pair-sum only; the final pair add happens on the host
                    # in f32 (trades a 1x DVE op for a 2x-wider outn store)
                    non = op.tile([P, nv, H, 2], mybir.dt.bfloat16, tag="non")
                    veng.tensor_tensor(
                        out=non[:],
                        in0=wv[:, :, :, 0:2], in1=wv[:, :, :, 2:4],
                        op=mybir.AluOpType.add,
                    )
                    wfin = non
                else:
                    wfin = wv
                nc.sync.dma_start(outn[:, ob : ob + nv, :, :],
                                  wfin[:, :, :, 0:2])
                nc.sync.dma_start(outd[:, ob : ob + nv, :], ed[:])

            emit.first = True
            emit.tiles = None
            for rep in range(reps):
                ob0 = (rep % OR) * NVT if bench_outs else 0
                # pool-assigned work is emitted FIRST: DMA issue order gates
                # when the Pool engine can start, and Pool runs ~3x slower
                # per slot, so it needs the head start to overlap DVE
                if pool_pairs:
                    for a in range(0, Sp, Bp):
                        w = min(Bp, Sp - a)
                        emit(2, kqp, vsp, w // 2, a // 2 + SH * rep,
                             ob0 + NVq + a // 2, veng=nc.gpsimd)
                for i, a in enumerate(range(0, Sq, Bw)):
                    w = min(Bw, Sq - a)
                    ve = nc.gpsimd if i < pool_qblocks else None
                    emit(4, kqs, vsi, w // 4, a // 4 + SH * rep,
                         ob0 + a // 4, veng=ve)
                if not pool_pairs:
                    for a in range(0, Sp, Bp):
                        w = min(Bp, Sp - a)
                        emit(2, kqp, vsp, w // 2, a // 2 + SH * rep,
                             ob0 + NVq + a // 2)
    nc.compile()
    return nc


# ================================================================ host build
def _stream_phase(slotdst, qvnode, g, Kh, Vb, Qloc):
    """One phase's streams: kq [P,NV,g+1,32] fp16 (g K rows + q row per
    group), vs [P,NV,32,g] bf16 (group-interleaved V)."""
    S = slotdst.shape[1]
    NV = S // g
    real = slotdst >= 0

    kss = np.zeros((P, S, H), np.float16)
    kss[real] = Kh[slotdst[real]]
    vss = np.zeros((P, S, H), BF16)
    vss[real] = Vb[slotdst[real]]
    vs = np.ascontiguousarray(vss.reshape(P, NV, g, H).transpose(0, 1, 3, 2))

    # pads sharing a group with a real node: poison-K so exp(score) ~ 0
    qvn = np.repeat(qvnode, g, axis=1)
    padm = (~real) & (qvn >= 0)
    if padm.any():
        q = Qloc[qvn[padm]].astype(np.float32)
        kpad = (-200.0 / np.maximum((q * q).sum(1), 1e-9))[:, None] * q
        kss[padm] = kpad.astype(np.float16)

    kq = np.zeros((P, NV, g + 1, H), np.float16)
    kq[:, :, 0:g, :] = kss.reshape(P, NV, g, H)
    validv = qvnode >= 0
    kq[:, :, g, :][validv] = Qloc[qvnode[validv]]
    return kq, vs


def _combine(cc, outn, outd):
    """Per-node segment reduction of group partials; returns [NPC, H].
    outn rows: [quad groups | pair groups] per partition."""
    qcat = np.concatenate([cc["qvnode_q"], cc["qvnode_p"]], axis=1).ravel()
    valid = qcat >= 0
    idx = qcat[valid]
    num = outn.reshape(-1, H, 2)[valid].astype(np.float32).sum(-1)
    den = outd.reshape(-1)[valid].astype(np.float32)
    # groups of one node are contiguous within a region, but a node may have
    # a quad run AND a pair row -> accumulate run sums per node
    starts = np.flatnonzero(np.diff(idx, prepend=idx[0] - 1) != 0)
    accn = np.add.reduceat(num, starts, axis=0)
    accd = np.add.reduceat(den, starts)
    nsum = np.zeros((NPC, H), np.float32)
    dsum = np.zeros(NPC, np.float32)
    np.add.at(nsum, idx[starts], accn)
    np.add.at(dsum, idx[starts], accd)
    dsum[dsum == 0] = 1.0
    return nsum / dsum[:, None]


# ================================================================ driver
def kernel(X, edge_index, Wq, Wk, Wv):
    X = np.ascontiguousarray(np.asarray(X, dtype=np.float32))
    Wq = np.asarray(Wq, dtype=np.float32)
    Wk = np.asarray(Wk, dtype=np.float32)
    Wv = np.asarray(Wv, dtype=np.float32)
    ei = np.asarray(edge_index)

    global LAST_S
    cores, Sq, Sp = _prep(ei)
    LAST_S = (Sq, Sp)

    # ---- kernel 1: projections
    if "k1" not in _cache:
        _cache["k1"] = _build_k1()
    k1 = _cache["k1"]
    w_cat = np.concatenate([Wq, Wk, Wv], axis=1).astype(np.float16)  # [256, 96]
    in1 = [
        {
            "xt": np.ascontiguousarray(X[c * NPC : (c + 1) * NPC].T).astype(
                np.float16
            ),
            "w": w_cat,
        }
        for c in range(NCORES)
    ]
    r1 = run_bass_kernel_spmd(k1, in1, core_ids=list(range(NCORES)))
    LAST_TIMES["k1"] = r1.exec_time_ns
    # qkv comes back transposed: [96, NPC] per core
    qkvT = [r1.results[c]["qkv"] for c in range(NCORES)]
    Kh = np.ascontiguousarray(
        np.concatenate([q[H : 2 * H, :].T for q in qkvT], axis=0)
    )  # [N, 32] fp16
    Vb = np.concatenate(
        [q[2 * H :, :].T for q in qkvT], axis=0
    ).astype(BF16)

    # ---- kernel 2: stream slots, edge compute, group partials
    if ("k2", Sq, Sp) not in _cache:
        _cache[("k2", Sq, Sp)] = _build_k2(Sq, Sp)
    k2 = _cache[("k2", Sq, Sp)]
    in2 = []
    for c in range(NCORES):
        Qloc = np.ascontiguousarray(qkvT[c][:H, :].T)
        kqs, vsi = _stream_phase(
            cores[c]["slotdst_q"], cores[c]["qvnode_q"], 4, Kh, Vb, Qloc
        )
        kqp, vsp = _stream_phase(
            cores[c]["slotdst_p"], cores[c]["qvnode_p"], 2, Kh, Vb, Qloc
        )
        in2.append({"kqs": kqs, "vsi": vsi, "kqp": kqp, "vsp": vsp})
    r2 = run_bass_kernel_spmd(k2, in2, core_ids=list(range(NCORES)))
    LAST_TIMES["k2"] = r2.exec_time_ns

    # ---- host combine
    out = np.empty((N, H), dtype=np.float32)
    for c in range(NCORES):
        out[c * NPC : (c + 1) * NPC] = _combine(
            cores[c], r2.results[c]["outn"], r2.results[c]["outd"]
        )
    return out


# revision 25
# speedup vs baseline: 1.2934x; 1.0201x over previous
"""Trainium2 Bass kernel for nn_MemoryAggregator (GNN attention aggregation).

Reference computation:
    Q = X@Wq; K = X@Wk; V = X@Wv            (X [100000,256], W [256,32])
    scores_e = <Q[src_e], K[dst_e]> / sqrt(32)   over 1.6M edges
    out[n]   = softmax-weighted sum over n's edges of V[dst_e]   ([100000,32])

Strategy (8 NeuronCores, SPMD, edges sharded by src node range):
  kernel1: per-core QKV projections of the core's 12500-node X shard.
           W^T stationary on the PE ([128,2,96] fp16), X^T streamed in 5
           chunks (2500 cols each, 5 psum tiles of 500, per-chunk output
           stores on the ACT queue); output written TRANSPOSED as
           qkv [96, 12500] fp16. DMA-bound (~8.8MB/core, ~25us).
  host:    two-phase slot streams per partition row: QUADS (4 slots share
           one q row + one numerator pair) for d//4 groups per node, plus
           one tail PAIR for d%4 in {1,2} (d%4==3 rounds into a quad).
           Greedy best-fit-decreasing packs nodes onto the 128 partitions
           (~1636 slots/partition vs 1562.5 ideal). Pad slots get
           K = -200*q/|q|^2 so their score is ~-35 and exp underflows.
  kernel2: per core, stream blocks and compute on the DVE only (GPSIMD
           tensor ops measured 3-4x slower than the cost model; every
           offload attempt regressed): q*k products then a pairwise
           tensor_tensor add chain (all 2x mode; TensorReduce runs 1x and
           is avoided), exp on ACT, ex*v + pair-sum (bf16). Writes
           PER-PAIR numerators [H,2] (the last 1x add is pushed to the
           host) + group-summed exp. ~115 DVE-ns/slot vs ~105 DMA-ns.
  host:    per-node segment reduction (f32) of pair partials + division.

Softmax max-subtraction is dropped: scores/sqrt(32) ~ N(0,4), max over 1.6M
edges ~21, exp safe in f32/bf16 (fp16 scores, bf16 exp pipeline validated
well under the 2e-2 tolerance).

Measured on the axon trn2 pool (PJRT wall-clock rep-differencing, see
test.py): baseline 187.2us total -> k1 25.5us + k2 ~105us.
"""
import math
from contextlib import ExitStack

import numpy as np

import concourse.bass as bass
import concourse.tile as tile
from concourse import bacc, mybir
from concourse.bass_utils import run_bass_kernel_spmd

# ---------------------------------------------------------------- dimensions
N = 100000
E = 1600000
D_IN = 256
H = 32
DK = math.sqrt(H)
NCORES = 8
NPC = N // NCORES          # 12500 nodes per core
P = 128
G = 4                      # slots per group (one q row / numerator per group)
B = 240                    # slot block width per k2 step (multiple of G)

BF16 = mybir.dt.np(mybir.dt.bfloat16)

_cache = {}
LAST_TIMES = {}
LAST_S = None


# ================================================================ host prep
def _prep_core(src_l, dst):
    order = np.argsort(src_l, kind="stable")
    dst_s = dst[order].astype(np.int32)

    d = np.bincount(src_l, minlength=NPC)
    r = d % 4
    v4 = d // 4 + (r == 3)          # quad groups (d%4==3 rounds into a quad)
    v2 = ((r == 1) | (r == 2)).astype(np.int64)   # one tail pair
    sq = 4 * v4
    sp = 2 * v2
    s = sq + sp

    # greedy best-fit-decreasing on total slots per partition
    node_order = np.argsort(-s, kind="stable")
    load = np.zeros(P, np.int64)
    part = np.empty(NPC, np.int64)
    for n in node_order:
        p = int(np.argmin(load))
        part[n] = p
        load[p] += s[n]
    loadq = np.bincount(part, weights=sq, minlength=P)
    loadp = np.bincount(part, weights=sp, minlength=P)
    return {"d": d, "v4": v4, "v2": v2, "part": part, "dst_s": dst_s,
            "Sq_core": int(loadq.max()), "Sp_core": int(loadp.max())}


def _finalize_core(cc, Sq, Sp):
    d, v4, v2, part = cc["d"], cc["v4"], cc["v2"], cc["part"]
    perm = np.lexsort((np.arange(NPC), part))
    part_sorted = part[perm]
    pstart = np.searchsorted(part_sorted, np.arange(P))

    def offsets(sizes):
        szp = sizes[perm]
        cs = np.cumsum(szp) - szp
        base = cs[np.minimum(pstart, NPC - 1)]
        within = cs - base[part_sorted]
        w = np.empty(NPC, np.int64)
        w[perm] = within
        return w

    oq = offsets(4 * v4)
    op_ = offsets(2 * v2)
    qcap = np.minimum(d, 4 * v4)    # edges that land in the quad region

    nodes_rep = np.repeat(np.arange(NPC), d)
    ranks = np.arange(int(d.sum())) - np.repeat(np.cumsum(d) - d, d)
    inq = ranks < qcap[nodes_rep]
    slotdst_q = np.full((P, Sq), -1, np.int32)
    nq, rq = nodes_rep[inq], ranks[inq]
    slotdst_q[part[nq], oq[nq] + rq] = cc["dst_s"][inq]
    slotdst_p = np.full((P, Sp), -1, np.int32)
    npr, rp = nodes_rep[~inq], ranks[~inq]
    slotdst_p[part[npr], op_[npr] + rp - qcap[npr]] = cc["dst_s"][~inq]

    qvnode_q = np.full((P, Sq // 4), -1, np.int32)
    vrep = np.repeat(np.arange(NPC), v4)
    vranks = np.arange(int(v4.sum())) - np.repeat(np.cumsum(v4) - v4, v4)
    qvnode_q[part[vrep], oq[vrep] // 4 + vranks] = vrep
    qvnode_p = np.full((P, Sp // 2), -1, np.int32)
    wn = np.flatnonzero(v2)
    qvnode_p[part[wn], op_[wn] // 2] = wn

    cc["slotdst_q"] = slotdst_q
    cc["qvnode_q"] = qvnode_q
    cc["slotdst_p"] = slotdst_p
    cc["qvnode_p"] = qvnode_p
    del cc["dst_s"], cc["d"], cc["v4"], cc["v2"], cc["part"]


def _prep(edge_index):
    src = np.asarray(edge_index[0], dtype=np.int64)
    dst = np.asarray(edge_index[1], dtype=np.int64)
    core = src // NPC
    cores = []
    for c in range(NCORES):
        m = core == c
        cores.append(_prep_core(src[m] - c * NPC, dst[m]))
    Sq = max(cc["Sq_core"] for cc in cores)
    Sq = (Sq + 3) // 4 * 4
    Sp = max(max(cc["Sp_core"] for cc in cores), 2)
    Sp = (Sp + 1) // 2 * 2
    for cc in cores:
        _finalize_core(cc, Sq, Sp)
    return cores, Sq, Sp


# ================================================================ kernel 1
K1CH = 5                   # X^T chunks (overlap DMA with matmul)
K1T = 5                    # psum tiles per chunk
K1C = NPC // (K1CH * K1T)  # 500 columns per psum tile


def _build_k1(reps=1, bench_outs=False, out_rot=None, mode="full"):
    # bench mode: per-rep input shift (defeats CSE) + rotating live outputs
    OR = (min(reps, out_rot) if out_rot else reps) if bench_outs else 1
    SH = 2 if bench_outs else 0
    nc = bacc.Bacc("TRN2", target_bir_lowering=False)
    xt = nc.dram_tensor(
        "xt", [D_IN, NPC + SH * reps], mybir.dt.float16, kind="ExternalInput"
    )
    w = nc.dram_tensor("w", [D_IN, 3 * H], mybir.dt.float16, kind="ExternalInput")
    qkv = nc.dram_tensor(
        "qkv", [3 * H, OR * NPC], mybir.dt.float16, kind="ExternalOutput"
    )

    csz = NPC // K1CH
    with tile.TileContext(nc) as tc:
        with ExitStack() as ctx:
            wp = ctx.enter_context(tc.tile_pool(name="wp", bufs=1))
            xp = ctx.enter_context(tc.tile_pool(name="xp", bufs=4))
            pp = ctx.enter_context(tc.tile_pool(name="pp", bufs=4, space="PSUM"))
            op = ctx.enter_context(tc.tile_pool(name="op", bufs=2))
            w01 = wp.tile([P, 2, 3 * H], mybir.dt.float16, tag="w01")
            nc.sync.dma_start(w01[:], w.rearrange("(g p) e -> p g e", g=2))
            xc0 = None
            if mode == "compute":
                # one resident chunk; every rep recomputes from it
                xc0 = wp.tile([P, 2, csz], mybir.dt.float16, tag="xc0")
                nc.sync.dma_start(
                    xc0[:], xt[:, 0:csz].rearrange("(g p) n -> p g n", g=2)
                )
            for rep in range(reps):
                i0 = SH * rep
                o0 = (rep % OR) * NPC if bench_outs else 0
                obuf = op.tile([3 * H, NPC], mybir.dt.float16, tag="obuf")
                if mode == "dma":
                    nc.vector.memset(obuf[:, 0:1], 0.0)
                for ch in range(K1CH):
                    c0 = ch * csz
                    if mode == "compute":
                        xc = xc0
                    else:
                        # partition p holds X^T rows p and p+128 of the chunk
                        xc = xp.tile([P, 2, csz], mybir.dt.float16, tag="xc")
                        nc.sync.dma_start(
                            xc[:],
                            xt[:, i0 + c0 : i0 + c0 + csz].rearrange(
                                "(g p) n -> p g n", g=2
                            ),
                        )
                        if mode == "dma":
                            continue
                    for t in range(K1T):
                        r0 = t * K1C
                        ps = pp.tile([3 * H, K1C], mybir.dt.float32, tag="ps")
                        nc.tensor.matmul(
                            ps[:], w01[:, 0, :], xc[:, 0, r0 : r0 + K1C],
                            start=True, stop=False,
                        )
                        nc.tensor.matmul(
                            ps[:], w01[:, 1, :], xc[:, 1, r0 : r0 + K1C],
                            start=False, stop=True,
                        )
                        dst = obuf[:, c0 + r0 : c0 + r0 + K1C]
                        if t % 2 == 0:
                            nc.vector.tensor_copy(dst, ps[:])
                        else:
                            nc.scalar.activation(
                                dst, ps[:], mybir.ActivationFunctionType.Copy
                            )
                    if mode != "dma":
                        # per-chunk store on the ACT queue overlaps the next
                        # chunk's load on the SP queue
                        nc.scalar.dma_start(
                            qkv[:, o0 + c0 : o0 + c0 + csz],
                            obuf[:, c0 : c0 + csz],
                        )
                if mode == "dma":
                    nc.sync.dma_start(qkv[:, o0 : o0 + NPC], obuf[:])
    nc.compile()
    return nc


# ================================================================ kernel 2
def _build_k2(Sq, Sp, reps=1, bench_outs=False, blk=None, blkp=None,
              dma_split=True, mode="full", out_rot=None,
              pool_pairs=False, pool_qblocks=0, alias=False, nwide=3):
    """Two-phase edge kernel: quad groups (4 slots share one q row and one
    numerator) then tail pairs (2 slots). All reductions are pairwise
    tensor_tensor add chains (2x DVE mode); TensorReduce (1x) is avoided.
    Slots of one node are contiguous within a partition row."""
    Bw = blk or B
    Bp = blkp or (B // 2)
    NVq = Sq // 4
    NVp = Sp // 2
    NVT = NVq + NVp
    # bench mode: per-rep input shift (defeats CSE) + rotating live outputs
    OR = (min(reps, out_rot) if out_rot else reps) if bench_outs else 1
    SH = 2 if bench_outs else 0
    nc = bacc.Bacc("TRN2", target_bir_lowering=False)
    kqs = nc.dram_tensor(
        "kqs", [P, NVq + SH * reps, 5, H], mybir.dt.float16,
        kind="ExternalInput"
    )
    vsi = nc.dram_tensor(
        "vsi", [P, NVq + SH * reps, H, 4], mybir.dt.bfloat16,
        kind="ExternalInput"
    )
    kqp = nc.dram_tensor(
        "kqp", [P, NVp + SH * reps, 3, H], mybir.dt.float16,
        kind="ExternalInput"
    )
    vsp = nc.dram_tensor(
        "vsp", [P, NVp + SH * reps, H, 2], mybir.dt.bfloat16,
        kind="ExternalInput"
    )
    # first `nwide` quad blocks store per-pair numerators [H,2] (skips the
    # 1x final DVE add); the rest store [H] (fewer DMA bytes) -- the split
    # balances the DVE-time vs DMA-time budgets, both near 110ns/slot
    NVW = min(nwide * ((blk or B) // 4), NVq)
    NVN = NVT - NVW
    outw = nc.dram_tensor(
        "outw", [P, max(OR * NVW, 1), H, 2], mybir.dt.bfloat16,
        kind="ExternalOutput"
    )
    outn = nc.dram_tensor(
        "outn", [P, max(OR * NVN, 1), H], mybir.dt.bfloat16,
        kind="ExternalOutput"
    )
    outd = nc.dram_tensor(
        "outd", [P, OR * NVT, 1], mybir.dt.bfloat16, kind="ExternalOutput"
    )

    with tile.TileContext(nc) as tc:
        with ExitStack() as ctx:
            kp = ctx.enter_context(tc.tile_pool(name="kp", bufs=2))
            vp = ctx.enter_context(tc.tile_pool(name="vp", bufs=2))
            sp = ctx.enter_context(tc.tile_pool(name="sp", bufs=2))
            op = ctx.enter_context(tc.tile_pool(name="op", bufs=3))
            zt = None
            if mode == "dma":
                zp = ctx.enter_context(tc.tile_pool(name="zp", bufs=1))
                zt = zp.tile([P, Bw // 4, H, 2], mybir.dt.bfloat16, tag="zt")
                nc.vector.memset(zt[:], 0.0)
                ztd = zp.tile([P, Bw // 4, 1], mybir.dt.bfloat16, tag="ztd")
                nc.vector.memset(ztd[:], 0.0)
            dmae = nc.scalar if dma_split else nc.sync

            def emit(g, kq_d, vs_d, nv, iv, ob, obd, wide, veng=None):
                """One block of nv g-slot groups; veng picks the vector
                engine (DVE or GPSIMD) for the whole block's pipeline."""
                veng = veng or nc.vector
                if mode != "compute" or emit.first:
                    kqt = kp.tile([P, nv, g + 1, H], mybir.dt.float16,
                                  tag="kqt")
                    nc.sync.dma_start(kqt[:], kq_d[:, iv : iv + nv, :, :])
                    vst = vp.tile([P, nv, H, g], mybir.dt.bfloat16, tag="vst")
                    dmae.dma_start(vst[:], vs_d[:, iv : iv + nv, :, :])
                    emit.tiles = (kqt, vst)
                    emit.first = False
                else:
                    kqt, vst = emit.tiles
                if mode == "dma":
                    if wide:
                        nc.sync.dma_start(
                            outw[:, ob : ob + nv, :, :], zt[:, 0:nv, :, :])
                    else:
                        nc.sync.dma_start(
                            outn[:, ob : ob + nv, :], zt[:, 0:nv, :, 0])
                    nc.sync.dma_start(
                        outd[:, obd : obd + nv, :], ztd[:, 0:nv, :])
                    return
                k_ap = kqt[:, :, 0:g, :]
                qv = kqt[:, :, g : g + 1, :]
                # scores: q*k then pairwise-add chain down to 1
                if alias:
                    # in-place: products overwrite the K region (write region
                    # == read region, element-aligned: DVE reads precede the
                    # trailing writeback), chain collapses onto its low half
                    veng.tensor_tensor(
                        out=k_ap, in0=qv.to_broadcast([P, nv, g, H]),
                        in1=k_ap, op=mybir.AluOpType.mult,
                    )
                    wdt = H
                    while wdt > 1:
                        veng.tensor_tensor(
                            out=kqt[:, :, 0:g, 0 : wdt // 2],
                            in0=kqt[:, :, 0:g, 0 : wdt // 2],
                            in1=kqt[:, :, 0:g, wdt // 2 : wdt],
                            op=mybir.AluOpType.add,
                        )
                        wdt //= 2
                    sc = kqt[:, :, 0:g, 0:1].rearrange("p v t o -> p v (t o)")
                else:
                    pr = sp.tile([P, nv, g, H], mybir.dt.float16, tag="pr")
                    veng.tensor_tensor(
                        out=pr[:, :, 0:g, :],
                        in0=qv.to_broadcast([P, nv, g, H]),
                        in1=k_ap, op=mybir.AluOpType.mult,
                    )
                    cur, wdt = pr, H
                    while wdt > 1:
                        nxt = sp.tile([P, nv, g, wdt // 2], mybir.dt.float16,
                                      tag=f"pc{wdt}")
                        veng.tensor_tensor(
                            out=nxt[:, :, 0:g, :],
                            in0=cur[:, :, 0:g, 0 : wdt // 2],
                            in1=cur[:, :, 0:g, wdt // 2 : wdt],
                            op=mybir.AluOpType.add,
                        )
                        cur, wdt = nxt, wdt // 2
                    sc = cur[:, :, 0:g, 0:1].rearrange("p v t o -> p v (t o)")
                ex = sp.tile([P, nv, g], mybir.dt.bfloat16, tag="ex")
                nc.scalar.activation(
                    ex[:, :, 0:g], sc, mybir.ActivationFunctionType.Exp,
                    scale=1.0 / DK,
                )
                # group-summed denominator (pairwise adds)
                if g == 4:
                    e2 = sp.tile([P, nv, 2], mybir.dt.bfloat16, tag="e2")
                    veng.tensor_tensor(
                        out=e2[:], in0=ex[:, :, 0:2], in1=ex[:, :, 2:4],
                        op=mybir.AluOpType.add,
                    )
                    edi = e2
                else:
                    edi = ex
                ed = op.tile([P, nv, 1], mybir.dt.bfloat16, tag="ed")
                veng.tensor_tensor(
                    out=ed[:], in0=edi[:, :, 0:1], in1=edi[:, :, 1:2],
                    op=mybir.AluOpType.add,
                )
                # weighted V + group sum
                exb = (
                    ex[:, :, 0:g]
                    .rearrange("p v (o t) -> p v o t", o=1)
                    .to_broadcast([P, nv, H, g])
                )
                wv = sp.tile([P, nv, H, g], mybir.dt.bfloat16, tag="wv")
                veng.tensor_tensor(
                    out=wv[:, :, :, 0:g], in0=exb, in1=vst[:, :, :, 0:g],
                    op=mybir.AluOpType.mult,
                )
                if g == 4:
                    non2 = op.tile([P, nv, H, 2], mybir.dt.bfloat16,
                                   tag="non2")
                    veng.tensor_tensor(
                        out=non2[:],
                        in0=wv[:, :, :, 0:2], in1=wv[:, :, :, 2:4],
                        op=mybir.AluOpType.add,
                    )
                    wfin = non2
                else:
                    wfin = wv
                if wide:
                    # per-pair numerators; host does the final add in f32
                    nc.sync.dma_start(outw[:, ob : ob + nv, :, :],
                                      wfin[:, :, :, 0:2])
                else:
                    non = op.tile([P, nv, H], mybir.dt.bfloat16, tag="non")
                    veng.tensor_tensor(
                        out=non[:, :, :].rearrange("p v (h o) -> p v h o",
                                                   o=1),
                        in0=wfin[:, :, :, 0:1], in1=wfin[:, :, :, 1:2],
                        op=mybir.AluOpType.add,
                    )
                    nc.sync.dma_start(outn[:, ob : ob + nv, :], non[:, :, :])
                nc.sync.dma_start(outd[:, obd : obd + nv, :], ed[:])

            emit.first = True
            emit.tiles = None
            for rep in range(reps):
                obw0 = (rep % OR) * NVW if bench_outs else 0
                obn0 = (rep % OR) * NVN if bench_outs else 0
                obd0 = (rep % OR) * NVT if bench_outs else 0
                for i, a in enumerate(range(0, Sq, Bw)):
                    w = min(Bw, Sq - a)
                    vb = a // 4
                    wide = vb < NVW
                    ob = (obw0 + vb) if wide else (obn0 + vb - NVW)
                    emit(4, kqs, vsi, w // 4, vb + SH * rep, ob,
                         obd0 + vb, wide)
                for a in range(0, Sp, Bp):
                    w = min(Bp, Sp - a)
                    vb = a // 2
                    emit(2, kqp, vsp, w // 2, vb + SH * rep,
                         obn0 + NVq - NVW + vb, obd0 + NVq + vb, False)
    nc.compile()
    return nc


# ================================================================ host build
def _stream_phase(slotdst, qvnode, g, Kh, Vb, Qloc):
    """One phase's streams: kq [P,NV,g+1,32] fp16 (g K rows + q row per
    group), vs [P,NV,32,g] bf16 (group-interleaved V)."""
    S = slotdst.shape[1]
    NV = S // g
    real = slotdst >= 0

    kss = np.zeros((P, S, H), np.float16)
    kss[real] = Kh[slotdst[real]]
    vss = np.zeros((P, S, H), BF16)
    vss[real] = Vb[slotdst[real]]
    vs = np.ascontiguousarray(vss.reshape(P, NV, g, H).transpose(0, 1, 3, 2))

    # pads sharing a group with a real node: poison-K so exp(score) ~ 0
    qvn = np.repeat(qvnode, g, axis=1)
    padm = (~real) & (qvn >= 0)
    if padm.any():
        q = Qloc[qvn[padm]].astype(np.float32)
        kpad = (-200.0 / np.maximum((q * q).sum(1), 1e-9))[:, None] * q
        kss[padm] = kpad.astype(np.float16)

    kq = np.zeros((P, NV, g + 1, H), np.float16)
    kq[:, :, 0:g, :] = kss.reshape(P, NV, g, H)
    validv = qvnode >= 0
    kq[:, :, g, :][validv] = Qloc[qvnode[validv]]
    return kq, vs


def _combine(cc, outw, outn, outd):
    """Per-node segment reduction of group partials; returns [NPC, H].
    outn rows: [quad groups | pair groups] per partition."""
    qcat = np.concatenate([cc["qvnode_q"], cc["qvnode_p"]], axis=1).ravel()
    valid = qcat >= 0
    idx = qcat[valid]
    numw = outw.astype(np.float32).sum(-1)          # [P, NVW, H]
    nums = np.concatenate([numw, outn.astype(np.float32)], axis=1)
    num = nums.reshape(-1, H)[valid]
    den = outd.reshape(-1)[valid].astype(np.float32)
    # groups of one node are contiguous within a region, but a node may have
    # a quad run AND a pair row -> accumulate run sums per node
    starts = np.flatnonzero(np.diff(idx, prepend=idx[0] - 1) != 0)
    accn = np.add.reduceat(num, starts, axis=0)
    accd = np.add.reduceat(den, starts)
    nsum = np.zeros((NPC, H), np.float32)
    dsum = np.zeros(NPC, np.float32)
    np.add.at(nsum, idx[starts], accn)
    np.add.at(dsum, idx[starts], accd)
    dsum[dsum == 0] = 1.0
    return nsum / dsum[:, None]


# ================================================================ driver
def kernel(X, edge_index, Wq, Wk, Wv):
    X = np.ascontiguousarray(np.asarray(X, dtype=np.float32))
    Wq = np.asarray(Wq, dtype=np.float32)
    Wk = np.asarray(Wk, dtype=np.float32)
    Wv = np.asarray(Wv, dtype=np.float32)
    ei = np.asarray(edge_index)

    global LAST_S
    cores, Sq, Sp = _prep(ei)
    LAST_S = (Sq, Sp)

    # ---- kernel 1: projections
    if "k1" not in _cache:
        _cache["k1"] = _build_k1()
    k1 = _cache["k1"]
    w_cat = np.concatenate([Wq, Wk, Wv], axis=1).astype(np.float16)  # [256, 96]
    in1 = [
        {
            "xt": np.ascontiguousarray(X[c * NPC : (c + 1) * NPC].T).astype(
                np.float16
            ),
            "w": w_cat,
        }
        for c in range(NCORES)
    ]
    r1 = run_bass_kernel_spmd(k1, in1, core_ids=list(range(NCORES)))
    LAST_TIMES["k1"] = r1.exec_time_ns
    # qkv comes back transposed: [96, NPC] per core
    qkvT = [r1.results[c]["qkv"] for c in range(NCORES)]
    Kh = np.ascontiguousarray(
        np.concatenate([q[H : 2 * H, :].T for q in qkvT], axis=0)
    )  # [N, 32] fp16
    Vb = np.concatenate(
        [q[2 * H :, :].T for q in qkvT], axis=0
    ).astype(BF16)

    # ---- kernel 2: stream slots, edge compute, group partials
    if ("k2", Sq, Sp) not in _cache:
        _cache[("k2", Sq, Sp)] = _build_k2(Sq, Sp)
    k2 = _cache[("k2", Sq, Sp)]
    in2 = []
    for c in range(NCORES):
        Qloc = np.ascontiguousarray(qkvT[c][:H, :].T)
        kqs, vsi = _stream_phase(
            cores[c]["slotdst_q"], cores[c]["qvnode_q"], 4, Kh, Vb, Qloc
        )
        kqp, vsp = _stream_phase(
            cores[c]["slotdst_p"], cores[c]["qvnode_p"], 2, Kh, Vb, Qloc
        )
        in2.append({"kqs": kqs, "vsi": vsi, "kqp": kqp, "vsp": vsp})
    r2 = run_bass_kernel_spmd(k2, in2, core_ids=list(range(NCORES)))
    LAST_TIMES["k2"] = r2.exec_time_ns

    # ---- host combine
    out = np.empty((N, H), dtype=np.float32)
    for c in range(NCORES):
        out[c * NPC : (c + 1) * NPC] = _combine(
            cores[c], r2.results[c]["outw"], r2.results[c]["outn"],
            r2.results[c]["outd"]
        )
    return out
